# revision 30
# baseline (speedup 1.0000x reference)
"""AdaEquiLayerNorm on Trainium2 — 8 NeuronCores, data-parallel over nodes.

Reference computation (N=100000 nodes, B=1024 graphs):
    emb   = [cos(t*f) | sin(t*f)]                  [B, 256]
    t_emb = silu(emb @ w1 + b1) @ w2 + b2          [B, 512]
    mod   = silu(t_emb[batch]) @ wm + bm           [N, 352]
    out   = per-irrep normalization of node_input modulated by mod

Key algebraic facts exploited here:
  * gather commutes with row-wise ops:  silu(t_emb[batch]) @ wm = (silu(t_emb) @ wm)[batch]
    so the big [N,512]x[512,352] matmul collapses to a [B,512]x[512,131] one
    plus a per-node row gather.
  * the reference only uses dyn_scale columns 0,1,2 (iw indexes irreps, not
    channels) and dyn_shift columns 0..127 — i.e. 131 of the 352 mod columns.

Per-core kernel (SPMD, identical graph on all 8 cores):
  1. compute the [1024, 256]-bf16 "mod table" (cols: s0 s1 s2 shift[128] pad)
     on-device with bf16 matmuls (replicated on every core; it is tiny),
  2. issue all per-node table-row gathers (dma_gather) as soon as the table
     lands in DRAM — the Q7 descriptor generation streams in the background,
  3. stream node super-tiles, software-pipelined: stats of tile k are emitted
     before the apply of tile k-1 so no engine head-of-line-blocks on the
     cross-engine stats->sqrt->apply chain.

Node <-> partition mapping inside a super-tile of 896 nodes: node (p, j) =
base + p*7 + j, i.e. each partition holds 7 CONSECUTIVE rows => every DMA
descriptor moves 7*480*4 = 13.4 KB contiguous (line-rate), and the host
permutes the gather index array to match dma_gather's fixed slot->partition
layout.

Sharding: cores 0..6 take rows [i*12544, (i+1)*12544); core 7 takes the last
12544 rows (overlapping core 6 by 352 rows so every shard is 98 tiles of 128).
"""

import sys
from contextlib import ExitStack

import numpy as np

try:
    import concourse.bass as bass
except ImportError:  # pragma: no cover
    sys.path.insert(0, "/opt/trn_rl_repo")
    import concourse.bass as bass

import concourse.mybir as mybir
import concourse.tile as tile
from concourse.bacc import Bacc
from concourse.tile_rust import add_dep_helper
from concourse.bass_utils import run_bass_kernel_spmd

F32 = mybir.dt.float32
BF16 = mybir.dt.bfloat16
I16 = mybir.dt.int16
AF = mybir.ActivationFunctionType
ALU = mybir.AluOpType

N_FULL = 100000
D_IN = 480            # 128 (l=0) + 192 (64x l=1) + 160 (32x l=2)
B = 1024
TIME = 512
N_CORES = 8
PER_CORE = 12544      # 98 tiles of 128 nodes
T_TILES = 7           # node rows per partition per super-tile (896 nodes/st)
EPS = 1e-5
MAGIC = 12582912.0    # 1.5 * 2^23 — fp32 add/sub rounds to nearest integer
TWO_PI = float(2.0 * np.pi)
TBL_W = 256           # bf16 table row: [s0 s1 s2 | shift(128) | zeros(125)]


def _bcast(ap_slice: bass.AP, count: int) -> bass.AP:
    """[.., 1] slice -> [.., count] via a stride-0 innermost dim."""
    a = [list(x) for x in ap_slice.ap]
    assert a[-1][1] == 1, a
    a[-1] = [0, count]
    return bass.AP(tensor=ap_slice.tensor, offset=ap_slice.offset, ap=a)


def build_nc(
    n_nodes: int = PER_CORE, t_tiles: int = T_TILES, native_silu: bool = True
) -> bass.Bass:
    stn = t_tiles * 128           # nodes per super-tile
    stw = stn // 16               # idx columns per super-tile
    assert n_nodes % stn == 0
    n_st = n_nodes // stn

    # Bacc (not raw Bass): its compile() pass legalizes multi-wait sync_info
    # into EVENT_SEMAPHORE instructions, auto-inserts the GPSIMD library
    # reloads dma_gather needs, and codegens extended-inst ISA bytes.
    nc = Bacc()
    x_ext = nc.declare_dram_parameter("node_input", [n_nodes, D_IN], F32, isOutput=False)
    idx_ext = nc.declare_dram_parameter("idx", [128, n_nodes // 16], I16, isOutput=False)
    t_ext = nc.declare_dram_parameter("t", [B], F32, isOutput=False)
    w1_ext = nc.declare_dram_parameter("w1", [256, TIME], F32, isOutput=False)
    b1_ext = nc.declare_dram_parameter("b1", [TIME], F32, isOutput=False)
    w2_ext = nc.declare_dram_parameter("w2", [TIME, TIME], F32, isOutput=False)
    b2_ext = nc.declare_dram_parameter("b2", [TIME], F32, isOutput=False)
    wmp_ext = nc.declare_dram_parameter("wmp", [TIME, TBL_W], F32, isOutput=False)
    bmp_ext = nc.declare_dram_parameter("bmp", [TBL_W], F32, isOutput=False)
    out_ext = nc.declare_dram_parameter("out", [n_nodes, D_IN], F32, isOutput=True)

    table = nc.dram_tensor("mod_table", [B, TBL_W], BF16)

    freqs = np.exp(-np.log(10000.0) * np.arange(128, dtype=np.float64) / 128.0)
    f2pi_const = nc.inline_tensor(
        (freqs / (2.0 * np.pi)).astype(np.float32).reshape(128, 1), name="f2pi"
    )

    def bcast_part(handle_ap: bass.AP, parts: int = 128) -> bass.AP:
        return bass.AP(
            tensor=handle_ap.tensor,
            offset=handle_ap.offset,
            ap=[[0, parts]] + list(handle_ap.ap),
        )

    with tile.TileContext(nc) as tc, ExitStack() as ctx:
        const = ctx.enter_context(tc.tile_pool(name="const", bufs=1))
        psum = ctx.enter_context(tc.tile_pool(name="psum", bufs=2, space="PSUM"))
        xio = ctx.enter_context(tc.tile_pool(name="xio", bufs=8))
        gio = ctx.enter_context(tc.tile_pool(name="gio", bufs=8))
        sm = ctx.enter_context(tc.tile_pool(name="sm", bufs=4))

        # ---- constants / weights into SBUF (weights cast to bf16 via SWDGE) ----
        # order matters on the gpsimd queue: t_bc gates the emb chain -> first
        f2pi_sb = const.tile([128, 1], F32)
        nc.gpsimd.dma_start(out=f2pi_sb, in_=f2pi_const[:, :])
        t_bc = const.tile([128, B], F32)
        nc.gpsimd.dma_start(out=t_bc, in_=bcast_part(t_ext[:]))
        idx_sb = const.tile([128, n_nodes // 16], I16)
        nc.sync.dma_start(out=idx_sb, in_=idx_ext[:, :])
        w1_sb = const.tile([128, 2, TIME], BF16)
        nc.gpsimd.dma_start(out=w1_sb, in_=w1_ext[:, :].rearrange("(k p) d -> p k d", p=128))
        w2_sb = const.tile([128, 4, TIME], BF16)
        nc.gpsimd.dma_start(out=w2_sb, in_=w2_ext[:, :].rearrange("(k p) d -> p k d", p=128))
        wmp_sb = const.tile([128, 4, TBL_W], BF16)
        nc.gpsimd.dma_start(out=wmp_sb, in_=wmp_ext[:, :].rearrange("(k p) d -> p k d", p=128))
        b1_sb = const.tile([128, 4], F32)
        nc.sync.dma_start(out=b1_sb, in_=b1_ext[:].rearrange("(m p) -> p m", p=128))
        b2_sb = const.tile([128, 4], F32)
        nc.sync.dma_start(out=b2_sb, in_=b2_ext[:].rearrange("(m p) -> p m", p=128))
        bmp_row = const.tile([1, TBL_W], F32)
        nc.sync.dma_start(out=bmp_row, in_=bmp_ext[None, :])
        ones_sb = const.tile([1, 128], BF16)
        nc.vector.memset(ones_sb, 1.0)
        eps_sb = const.tile([128, 1], F32)
        nc.vector.memset(eps_sb, EPS)
        quarter_sb = const.tile([128, 1], F32)
        nc.vector.memset(quarter_sb, 0.25)
        # warm-up gather from real input data: absorbs the one-time Q7 'mlp'
        # library IRAM load (~6-9us) during the table stage, so the first
        # real gather starts right when the table lands
        zidx = const.tile([128, 8], I16)
        nc.vector.memset(zidx, 0)
        warm_out = const.tile([128, 1, 64], F32)
        nc.gpsimd.dma_gather(
            out_ap=warm_out[:],
            in_ap=w2_ext[:, 0:64],
            idxs_ap=zidx[:, :],
            num_idxs=128,
            num_idxs_reg=128,
            elem_size=64,
            elem_step=TIME,
        )

        # prefetch the first node super-tiles while the table is being built
        def x_view(st):
            rows = slice(st * stn, (st + 1) * stn)
            return x_ext[rows, :].rearrange("(p t) c -> p t c", t=t_tiles)

        x_tiles = {}
        for st in range(min(8, n_st)):
            x_tiles[st] = xio.tile([128, t_tiles, D_IN], F32, tag="x", name=f"x{st}")
            nc.sync.dma_start(out=x_tiles[st], in_=x_view(st))

        # ---- table stage (scoped pool; SBUF released before the main loop) ----
        last_act = [None]
        with tc.tile_pool(name="tbl", bufs=1) as tbl:
            # embT[h][j, b] = cos/sin(t[b]*freqs[j]), bf16, via range-reduced Sin
            embT = []
            for h, turn in ((0, 0.25), (1, 0.0)):  # emb = [cos | sin]
                m = tbl.tile([128, B], F32, tag="m")
                nc.vector.tensor_scalar(
                    out=m, in0=t_bc, scalar1=f2pi_sb, scalar2=turn,
                    op0=ALU.mult, op1=ALU.add,
                )
                r = tbl.tile([128, B], F32, tag="r")
                nc.vector.tensor_scalar_add(out=r, in0=m, scalar1=MAGIC)
                nc.vector.tensor_scalar_sub(out=r, in0=r, scalar1=MAGIC)
                nc.vector.tensor_sub(out=m, in0=m, in1=r)
                e = tbl.tile([128, B], BF16, tag=f"e{h}")
                nc.scalar.activation(out=e, in_=m, func=AF.Sin, scale=TWO_PI)
                embT.append(e)

            def silu_from_psum(out_ap, psum_ap, bias_ap):
                if native_silu:
                    last_act[0] = nc.scalar.activation(
                        out=out_ap, in_=psum_ap, func=AF.Silu, bias=bias_ap, scale=1.0
                    ).ins
                else:  # CoreSim fallback: silu(x) = x * sigmoid(x)
                    lin = sm.tile([128, 512], F32, tag="silu_lin")
                    nc.scalar.activation(
                        out=lin, in_=psum_ap, func=AF.Identity, bias=bias_ap, scale=1.0
                    )
                    sig = sm.tile([128, 512], F32, tag="silu_sig")
                    nc.scalar.activation(out=sig, in_=lin, func=AF.Sigmoid)
                    nc.vector.tensor_mul(out=out_ap, in0=lin, in1=sig)

            # s1 = silu(emb @ w1 + b1)^T   [512(4 ptiles), B], bf16
            s1 = tbl.tile([128, 4, B], BF16)
            for mi in range(4):
                for nb in range(B // 512):
                    ps = psum.tile([128, 512], F32, tag="mlp", bufs=4)
                    for k in range(2):
                        nc.tensor.matmul(
                            ps, w1_sb[:, k, mi * 128:(mi + 1) * 128],
                            embT[k][:, nb * 512:(nb + 1) * 512],
                            start=(k == 0), stop=(k == 1),
                        )
                    silu_from_psum(
                        s1[:, mi, nb * 512:(nb + 1) * 512], ps, b1_sb[:, mi:mi + 1]
                    )
            # s2 = silu(s1^T @ w2 + b2)^T  (= silu(t_emb), fused), bf16
            s2 = tbl.tile([128, 4, B], BF16)
            for mi in range(4):
                for nb in range(B // 512):
                    ps = psum.tile([128, 512], F32, tag="mlp", bufs=4)
                    for k in range(4):
                        nc.tensor.matmul(
                            ps, w2_sb[:, k, mi * 128:(mi + 1) * 128],
                            s1[:, k, nb * 512:(nb + 1) * 512],
                            start=(k == 0), stop=(k == 3),
                        )
                    silu_from_psum(
                        s2[:, mi, nb * 512:(nb + 1) * 512], ps, b2_sb[:, mi:mi + 1]
                    )
            # table rows: mod[b, :] = silu(t_emb)[b] @ wmp + bmp  (bf16 in DRAM)
            # bmp is added by a K=1 matmul against a ones row, so the DVE is
            # not on the table critical path at all.
            bmp_bf = tbl.tile([1, TBL_W], BF16)
            nc.vector.tensor_copy(out=bmp_bf, in_=bmp_row)
            for bc in range(B // 128):
                psm = psum.tile([128, TBL_W], F32, tag="mod")
                for k in range(4):
                    nc.tensor.matmul(
                        psm, s2[:, k, bc * 128:(bc + 1) * 128], wmp_sb[:, k, :],
                        start=(k == 0), stop=False,
                    )
                nc.tensor.matmul(psm, ones_sb, bmp_bf, start=False, stop=True)
                msb = sm.tile([128, TBL_W], BF16, tag="msb", bufs=8)
                # DVE copy: the ACT queue is contended (silus + main-loop
                # squares); the table store must not sit behind it
                nc.vector.tensor_copy(out=msb, in_=psm)
                nc.sync.dma_start(out=table[bc * 128:(bc + 1) * 128, :], in_=msb)

        # ---- all gathers up front: Q7 descriptor gen streams in background ----
        g_tiles = {}
        for st in range(n_st):
            g_tiles[st] = gio.tile(
                [128, t_tiles, TBL_W], BF16, tag="g", name=f"g{st}"
            )
            nc.gpsimd.dma_gather(
                out_ap=g_tiles[st][:],
                in_ap=table[:, :],
                idxs_ap=idx_sb[:, st * stw:(st + 1) * stw],
                num_idxs=stn,
                num_idxs_reg=stn,
                elem_size=TBL_W,
            )

        # ---- main loop, software-pipelined by one super-tile ----
        sc1 = 1.0 / np.sqrt(192.0)  # Square(x*sc) accumulates ssq/192 directly
        sc2 = 1.0 / np.sqrt(160.0)
        state = {}

        def emit_stats(st):
            if st not in x_tiles:
                x_tiles[st] = xio.tile([128, t_tiles, D_IN], F32, tag="x", name=f"x{st}")
                nc.sync.dma_start(out=x_tiles[st], in_=x_view(st))
            x_sb = x_tiles[st]
            st6 = sm.tile([128, t_tiles, 6], F32, tag="st6")
            for ti in range(t_tiles):
                nc.vector.bn_stats(out=st6[:, ti, :], in_=x_sb[:, ti, 0:128])
            # v4 = [mean, var, ssq1/192, ssq2/160]; squares use y as scratch
            v4 = sm.tile([128, t_tiles, 4], F32, tag="v4", bufs=8)
            for ti in range(t_tiles):
                nc.vector.bn_aggr(out=v4[:, ti, 0:2], in_=st6[:, ti, :])
            def act_after_table(inst):
                # keep the ACT queue clear for the table stage: the gathers
                # (and thus the whole pipeline) wait on its completion
                if last_act[0] is not None:
                    add_dep_helper(
                        inst.ins, last_act[0], sync=False,
                        reason="stats ACT yields to table stage",
                    )
                return inst

            for ti in range(t_tiles):
                sq1 = sm.tile([128, 192], F32, tag="sq1")
                act_after_table(nc.scalar.activation(
                    out=sq1, in_=x_sb[:, ti, 128:320],
                    func=AF.Square, scale=sc1, accum_out=v4[:, ti, 2:3],
                ))
                sq2 = sm.tile([128, 160], F32, tag="sq2")
                act_after_table(nc.scalar.activation(
                    out=sq2, in_=x_sb[:, ti, 320:480],
                    func=AF.Square, scale=sc2, accum_out=v4[:, ti, 3:4],
                ))
            rr = sm.tile([128, t_tiles, 3], F32, tag="rr", bufs=8)
            act_after_table(nc.scalar.activation(
                out=rr, in_=v4[:, :, 1:4], func=AF.Sqrt, bias=eps_sb))
            nc.vector.reciprocal(out=rr, in_=rr)  # rstd0, rnorm1, rnorm2
            state[st] = (x_sb, v4, rr)

        def emit_apply(st):
            x_sb, v4, rr = state.pop(st)
            g = g_tiles.pop(st)
            # gather-dependent per-node multipliers live in the apply phase so
            # a late gather never head-of-line-blocks the next tile's stats
            sp1 = sm.tile([128, t_tiles, 3], F32, tag="sp1", bufs=8)
            nc.vector.tensor_scalar_add(out=sp1, in0=g[:, :, 0:3], scalar1=1.0)
            amul = sm.tile([128, t_tiles, 3], F32, tag="amul", bufs=8)
            nc.vector.tensor_mul(out=amul, in0=rr, in1=sp1)
            bmn = sm.tile([128, t_tiles, 1], F32, tag="bmn", bufs=8)
            nc.vector.tensor_mul(out=bmn, in0=v4[:, :, 0:1], in1=amul[:, :, 0:1])
            # apply IN-PLACE on the x tile: no y buffers, so x slots recycle
            # right after the store and the input DMA streams at wire rate
            if True:
                nc.vector.tensor_tensor(
                    out=x_sb[:, :, 128:320], in0=x_sb[:, :, 128:320],
                    in1=_bcast(amul[:, :, 1:2], 192), op=ALU.mult,
                )
                nc.vector.tensor_tensor(
                    out=x_sb[:, :, 320:480], in0=x_sb[:, :, 320:480],
                    in1=_bcast(amul[:, :, 2:3], 160), op=ALU.mult,
                )
                nc.vector.tensor_tensor(
                    out=x_sb[:, :, 0:128], in0=x_sb[:, :, 0:128],
                    in1=_bcast(amul[:, :, 0:1], 128), op=ALU.mult,
                )
                nc.vector.tensor_tensor(
                    out=x_sb[:, :, 0:128], in0=x_sb[:, :, 0:128],
                    in1=_bcast(bmn[:, :, 0:1], 128), op=ALU.subtract,
                )
            nc.vector.tensor_tensor(
                out=x_sb[:, :, 0:128], in0=x_sb[:, :, 0:128],
                in1=g[:, :, 3:131], op=ALU.add,
            )
            rows = slice(st * stn, (st + 1) * stn)
            nc.sync.dma_start(
                out=out_ext[rows, :].rearrange("(p t) c -> p t c", t=t_tiles),
                in_=x_sb,
            )

        for st in range(n_st + 1):
            if st < n_st:
                emit_stats(st)
            if st >= 1:
                emit_apply(st - 1)

    nc.finalize()  # Bacc.finalize runs compile(): sem legalization, lib loads
    return nc


def _prep_in_maps(node_input, t, batch, w1, b1, w2, b2, wm, bm, n_nodes=PER_CORE,
                  t_tiles=T_TILES):
    stn = t_tiles * 128
    n_st = n_nodes // stn
    wmp = np.zeros((TIME, TBL_W), np.float32)
    wmp[:, 0:3] = wm[:, 0:3]
    wmp[:, 3:131] = wm[:, 224:352]
    bmp = np.zeros((TBL_W,), np.float32)
    bmp[0:3] = bm[0:3]
    bmp[3:131] = bm[224:352]
    shared = {
        "t": np.ascontiguousarray(t, dtype=np.float32),
        "w1": np.ascontiguousarray(w1, dtype=np.float32),
        "b1": np.ascontiguousarray(b1, dtype=np.float32),
        "w2": np.ascontiguousarray(w2, dtype=np.float32),
        "b2": np.ascontiguousarray(b2, dtype=np.float32),
        "wmp": wmp,
        "bmp": bmp,
    }
    n = node_input.shape[0]
    starts = [min(i * n_nodes, n - n_nodes) for i in range(N_CORES)]
    in_maps = []
    for s in starts:
        sl = slice(s, s + n_nodes)
        # node (st, p, j) = base + p*t_tiles + j; dma_gather writes slot
        # i = j*128 + p of super-tile st to [p, st, j, :], and reads slot i's
        # index from idx[(i%16) + 16k, st*stw + i//16].
        ids = batch[sl].astype(np.int16).reshape(n_st, 128, t_tiles)
        perm = ids.transpose(0, 2, 1).reshape(n_st, stn)       # [st, j*128+p]
        # [16, n_st*stw] with element [i%16, st*stw + i//16] = perm[st, i]
        cols = perm.reshape(n_st, stn // 16, 16)               # [st, c, r]
        idx16 = np.concatenate([cols[s2].T for s2 in range(n_st)], axis=1)
        idx = np.ascontiguousarray(np.tile(idx16, (8, 1)))
        in_maps.append(
            {
                **shared,
                "node_input": np.ascontiguousarray(node_input[sl], dtype=np.float32),
                "idx": idx,
            }
        )
    return in_maps, starts


_NC_CACHE: dict = {}


def _get_nc(n_nodes=PER_CORE, t_tiles=T_TILES):
    key = (n_nodes, t_tiles)
    if key not in _NC_CACHE:
        _NC_CACHE[key] = build_nc(n_nodes, t_tiles)
    return _NC_CACHE[key]


def run(node_input, t, batch, w1, b1, w2, b2, wm, bm, trace=False, **trace_kwargs):
    """Run on 8 NeuronCores; returns (full output, BassKernelResults)."""
    node_input = np.asarray(node_input)
    n = node_input.shape[0]
    in_maps, starts = _prep_in_maps(
        node_input, np.asarray(t), np.asarray(batch),
        np.asarray(w1), np.asarray(b1), np.asarray(w2), np.asarray(b2),
        np.asarray(wm), np.asarray(bm),
    )
    nc = _get_nc()
    res = run_bass_kernel_spmd(
        nc, in_maps, core_ids=list(range(N_CORES)), trace=trace, **trace_kwargs
    )
    out = np.empty((n, D_IN), dtype=np.float32)
    for s, core_res in zip(starts, res.results):
        out[s:s + PER_CORE] = core_res["out"]
    return out, res


def kernel(node_input, t, batch, w1, b1, w2, b2, wm, bm):
    out, _ = run(node_input, t, batch, w1, b1, w2, b2, wm, bm, trace=False)
    return out


# revision 32
# speedup vs baseline: 1.0925x; 1.0925x over previous
"""AdaEquiLayerNorm on Trainium2 — 8 NeuronCores, data-parallel over nodes.

Reference computation (N=100000 nodes, B=1024 graphs):
    emb   = [cos(t*f) | sin(t*f)]                  [B, 256]
    t_emb = silu(emb @ w1 + b1) @ w2 + b2          [B, 512]
    mod   = silu(t_emb[batch]) @ wm + bm           [N, 352]
    out   = per-irrep normalization of node_input modulated by mod

Key algebraic facts exploited here:
  * gather commutes with row-wise ops:  silu(t_emb[batch]) @ wm = (silu(t_emb) @ wm)[batch]
    so the big [N,512]x[512,352] matmul collapses to a [B,512]x[512,131] one
    plus a per-node row gather.
  * the reference only uses dyn_scale columns 0,1,2 (iw indexes irreps, not
    channels) and dyn_shift columns 0..127 — i.e. 131 of the 352 mod columns.

Per-core kernel (SPMD, identical graph on all 8 cores):
  1. compute the [1024, 256]-bf16 "mod table" (cols: s0 s1 s2 shift[128] pad)
     on-device with bf16 matmuls (replicated on every core; it is tiny),
  2. issue all per-node table-row gathers (dma_gather) as soon as the table
     lands in DRAM — the Q7 descriptor generation streams in the background,
  3. stream node super-tiles, software-pipelined: stats of tile k are emitted
     before the apply of tile k-1 so no engine head-of-line-blocks on the
     cross-engine stats->sqrt->apply chain.

Node <-> partition mapping inside a super-tile of 896 nodes: node (p, j) =
base + p*7 + j, i.e. each partition holds 7 CONSECUTIVE rows => every DMA
descriptor moves 7*480*4 = 13.4 KB contiguous (line-rate), and the host
permutes the gather index array to match dma_gather's fixed slot->partition
layout.

Sharding: cores 0..6 take rows [i*12544, (i+1)*12544); core 7 takes the last
12544 rows (overlapping core 6 by 352 rows so every shard is 98 tiles of 128).
"""

import sys
from contextlib import ExitStack

import numpy as np

try:
    import concourse.bass as bass
except ImportError:  # pragma: no cover
    sys.path.insert(0, "/opt/trn_rl_repo")
    import concourse.bass as bass

import concourse.mybir as mybir
import concourse.tile as tile
from concourse.bacc import Bacc
from concourse.tile_rust import add_dep_helper
from concourse.bass_utils import run_bass_kernel_spmd

F32 = mybir.dt.float32
BF16 = mybir.dt.bfloat16
I16 = mybir.dt.int16
AF = mybir.ActivationFunctionType
ALU = mybir.AluOpType

N_FULL = 100000
D_IN = 480            # 128 (l=0) + 192 (64x l=1) + 160 (32x l=2)
B = 1024
TIME = 512
N_CORES = 8
PER_CORE = 12544      # 98 tiles of 128 nodes
T_TILES = 7           # node rows per partition per super-tile (896 nodes/st)
EPS = 1e-5
MAGIC = 12582912.0    # 1.5 * 2^23 — fp32 add/sub rounds to nearest integer
TWO_PI = float(2.0 * np.pi)
TBL_W = 256           # bf16 table row: [s0 s1 s2 | shift(128) | zeros(125)]


def _bcast(ap_slice: bass.AP, count: int) -> bass.AP:
    """[.., 1] slice -> [.., count] via a stride-0 innermost dim."""
    a = [list(x) for x in ap_slice.ap]
    assert a[-1][1] == 1, a
    a[-1] = [0, count]
    return bass.AP(tensor=ap_slice.tensor, offset=ap_slice.offset, ap=a)


def build_nc(
    n_nodes: int = PER_CORE, t_tiles: int = T_TILES, native_silu: bool = True
) -> bass.Bass:
    stn = t_tiles * 128           # nodes per super-tile
    stw = stn // 16               # idx columns per super-tile
    assert n_nodes % stn == 0
    n_st = n_nodes // stn

    # Bacc (not raw Bass): its compile() pass legalizes multi-wait sync_info
    # into EVENT_SEMAPHORE instructions, auto-inserts the GPSIMD library
    # reloads dma_gather needs, and codegens extended-inst ISA bytes.
    nc = Bacc()
    x_ext = nc.declare_dram_parameter("node_input", [n_nodes, D_IN], F32, isOutput=False)
    idx_ext = nc.declare_dram_parameter("idx", [128, n_nodes // 16], I16, isOutput=False)
    t_ext = nc.declare_dram_parameter("t", [B], F32, isOutput=False)
    w1_ext = nc.declare_dram_parameter("w1", [256, TIME], F32, isOutput=False)
    b1_ext = nc.declare_dram_parameter("b1", [TIME], F32, isOutput=False)
    w2_ext = nc.declare_dram_parameter("w2", [TIME, TIME], F32, isOutput=False)
    b2_ext = nc.declare_dram_parameter("b2", [TIME], F32, isOutput=False)
    wmp_ext = nc.declare_dram_parameter("wmp", [TIME, TBL_W], F32, isOutput=False)
    bmp_ext = nc.declare_dram_parameter("bmp", [TBL_W], F32, isOutput=False)
    out_ext = nc.declare_dram_parameter("out", [n_nodes, D_IN], F32, isOutput=True)

    table = nc.dram_tensor("mod_table", [B, TBL_W], BF16)

    freqs = np.exp(-np.log(10000.0) * np.arange(128, dtype=np.float64) / 128.0)
    f2pi_const = nc.inline_tensor(
        (freqs / (2.0 * np.pi)).astype(np.float32).reshape(128, 1), name="f2pi"
    )

    def bcast_part(handle_ap: bass.AP, parts: int = 128) -> bass.AP:
        return bass.AP(
            tensor=handle_ap.tensor,
            offset=handle_ap.offset,
            ap=[[0, parts]] + list(handle_ap.ap),
        )

    with tile.TileContext(nc) as tc, ExitStack() as ctx:
        const = ctx.enter_context(tc.tile_pool(name="const", bufs=1))
        psum = ctx.enter_context(tc.tile_pool(name="psum", bufs=2, space="PSUM"))
        xio = ctx.enter_context(tc.tile_pool(name="xio", bufs=8))
        gio = ctx.enter_context(tc.tile_pool(name="gio", bufs=8))
        sm = ctx.enter_context(tc.tile_pool(name="sm", bufs=4))

        # ---- constants / weights into SBUF (weights cast to bf16 via SWDGE) ----
        # order matters on the gpsimd queue: t_bc gates the emb chain -> first
        f2pi_sb = const.tile([128, 1], F32)
        nc.gpsimd.dma_start(out=f2pi_sb, in_=f2pi_const[:, :])
        t_bc = const.tile([128, B], F32)
        nc.gpsimd.dma_start(out=t_bc, in_=bcast_part(t_ext[:]))
        idx_sb = const.tile([128, n_nodes // 16], I16)
        nc.sync.dma_start(out=idx_sb, in_=idx_ext[:, :])
        w1_sb = const.tile([128, 2, TIME], BF16)
        nc.gpsimd.dma_start(out=w1_sb, in_=w1_ext[:, :].rearrange("(k p) d -> p k d", p=128))
        w2_sb = const.tile([128, 4, TIME], BF16)
        nc.gpsimd.dma_start(out=w2_sb, in_=w2_ext[:, :].rearrange("(k p) d -> p k d", p=128))
        wmp_sb = const.tile([128, 4, TBL_W], BF16)
        nc.gpsimd.dma_start(out=wmp_sb, in_=wmp_ext[:, :].rearrange("(k p) d -> p k d", p=128))
        b1_sb = const.tile([128, 4], F32)
        nc.sync.dma_start(out=b1_sb, in_=b1_ext[:].rearrange("(m p) -> p m", p=128))
        b2_sb = const.tile([128, 4], F32)
        nc.sync.dma_start(out=b2_sb, in_=b2_ext[:].rearrange("(m p) -> p m", p=128))
        bmp_row = const.tile([1, TBL_W], F32)
        nc.sync.dma_start(out=bmp_row, in_=bmp_ext[None, :])
        ones_sb = const.tile([1, 128], BF16)
        nc.vector.memset(ones_sb, 1.0)
        eps_sb = const.tile([128, 1], F32)
        nc.vector.memset(eps_sb, EPS)
        quarter_sb = const.tile([128, 1], F32)
        nc.vector.memset(quarter_sb, 0.25)

        # prefetch the first node super-tiles while the table is being built
        def x_view(st):
            rows = slice(st * stn, (st + 1) * stn)
            return x_ext[rows, :].rearrange("(p t) c -> p t c", t=t_tiles)

        x_tiles = {}
        for st in range(min(8, n_st)):
            x_tiles[st] = xio.tile([128, t_tiles, D_IN], F32, tag="x", name=f"x{st}")
            nc.sync.dma_start(out=x_tiles[st], in_=x_view(st))

        # ---- table stage (scoped pool; SBUF released before the main loop) ----
        last_act = [None]
        with tc.tile_pool(name="tbl", bufs=1) as tbl:
            # embT[h][j, b] = cos/sin(t[b]*freqs[j]), bf16, via range-reduced Sin
            embT = []
            for h, turn in ((0, 0.25), (1, 0.0)):  # emb = [cos | sin]
                m = tbl.tile([128, B], F32, tag="m")
                nc.vector.tensor_scalar(
                    out=m, in0=t_bc, scalar1=f2pi_sb, scalar2=turn,
                    op0=ALU.mult, op1=ALU.add,
                )
                r = tbl.tile([128, B], F32, tag="r")
                nc.vector.tensor_scalar_add(out=r, in0=m, scalar1=MAGIC)
                nc.vector.tensor_scalar_sub(out=r, in0=r, scalar1=MAGIC)
                nc.vector.tensor_sub(out=m, in0=m, in1=r)
                e = tbl.tile([128, B], BF16, tag=f"e{h}")
                nc.scalar.activation(out=e, in_=m, func=AF.Sin, scale=TWO_PI)
                embT.append(e)

            def silu_from_psum(out_ap, psum_ap, bias_ap):
                if native_silu:
                    last_act[0] = nc.scalar.activation(
                        out=out_ap, in_=psum_ap, func=AF.Silu, bias=bias_ap, scale=1.0
                    ).ins
                else:  # CoreSim fallback: silu(x) = x * sigmoid(x)
                    lin = sm.tile([128, 512], F32, tag="silu_lin")
                    nc.scalar.activation(
                        out=lin, in_=psum_ap, func=AF.Identity, bias=bias_ap, scale=1.0
                    )
                    sig = sm.tile([128, 512], F32, tag="silu_sig")
                    nc.scalar.activation(out=sig, in_=lin, func=AF.Sigmoid)
                    nc.vector.tensor_mul(out=out_ap, in0=lin, in1=sig)

            # s1 = silu(emb @ w1 + b1)^T   [512(4 ptiles), B], bf16
            s1 = tbl.tile([128, 4, B], BF16)
            for mi in range(4):
                for nb in range(B // 512):
                    ps = psum.tile([128, 512], F32, tag="mlp", bufs=4)
                    for k in range(2):
                        nc.tensor.matmul(
                            ps, w1_sb[:, k, mi * 128:(mi + 1) * 128],
                            embT[k][:, nb * 512:(nb + 1) * 512],
                            start=(k == 0), stop=(k == 1),
                        )
                    silu_from_psum(
                        s1[:, mi, nb * 512:(nb + 1) * 512], ps, b1_sb[:, mi:mi + 1]
                    )
            # s2 = silu(s1^T @ w2 + b2)^T  (= silu(t_emb), fused), bf16
            s2 = tbl.tile([128, 4, B], BF16)
            for mi in range(4):
                for nb in range(B // 512):
                    ps = psum.tile([128, 512], F32, tag="mlp", bufs=4)
                    for k in range(4):
                        nc.tensor.matmul(
                            ps, w2_sb[:, k, mi * 128:(mi + 1) * 128],
                            s1[:, k, nb * 512:(nb + 1) * 512],
                            start=(k == 0), stop=(k == 3),
                        )
                    silu_from_psum(
                        s2[:, mi, nb * 512:(nb + 1) * 512], ps, b2_sb[:, mi:mi + 1]
                    )
            # table rows: mod[b, :] = silu(t_emb)[b] @ wmp + bmp  (bf16 in DRAM)
            # bmp is added by a K=1 matmul against a ones row, so the DVE is
            # not on the table critical path at all.
            bmp_bf = tbl.tile([1, TBL_W], BF16)
            nc.vector.tensor_copy(out=bmp_bf, in_=bmp_row)
            for bc in range(B // 128):
                psm = psum.tile([128, TBL_W], F32, tag="mod")
                for k in range(4):
                    nc.tensor.matmul(
                        psm, s2[:, k, bc * 128:(bc + 1) * 128], wmp_sb[:, k, :],
                        start=(k == 0), stop=False,
                    )
                nc.tensor.matmul(psm, ones_sb, bmp_bf, start=False, stop=True)
                msb = sm.tile([128, TBL_W], BF16, tag="msb", bufs=8)
                # DVE copy: the ACT queue is contended (silus + main-loop
                # squares); the table store must not sit behind it
                nc.vector.tensor_copy(out=msb, in_=psm)
                nc.sync.dma_start(out=table[bc * 128:(bc + 1) * 128, :], in_=msb)

        # ---- all gathers up front: Q7 descriptor gen streams in background ----
        g_tiles = {}
        for st in range(n_st):
            g_tiles[st] = gio.tile(
                [128, t_tiles, TBL_W], BF16, tag="g", name=f"g{st}"
            )
            nc.gpsimd.dma_gather(
                out_ap=g_tiles[st][:],
                in_ap=table[:, :],
                idxs_ap=idx_sb[:, st * stw:(st + 1) * stw],
                num_idxs=stn,
                num_idxs_reg=stn,
                elem_size=TBL_W,
                single_packet=False,
            )

        # ---- main loop, software-pipelined by one super-tile ----
        sc1 = 1.0 / np.sqrt(192.0)  # Square(x*sc) accumulates ssq/192 directly
        sc2 = 1.0 / np.sqrt(160.0)
        state = {}

        def emit_stats(st):
            if st not in x_tiles:
                x_tiles[st] = xio.tile([128, t_tiles, D_IN], F32, tag="x", name=f"x{st}")
                nc.sync.dma_start(out=x_tiles[st], in_=x_view(st))
            x_sb = x_tiles[st]
            st6 = sm.tile([128, t_tiles, 6], F32, tag="st6")
            for ti in range(t_tiles):
                nc.vector.bn_stats(out=st6[:, ti, :], in_=x_sb[:, ti, 0:128])
            # v4 = [mean, var, ssq1/192, ssq2/160]; squares use y as scratch
            v4 = sm.tile([128, t_tiles, 4], F32, tag="v4", bufs=8)
            for ti in range(t_tiles):
                nc.vector.bn_aggr(out=v4[:, ti, 0:2], in_=st6[:, ti, :])
            def act_after_table(inst):
                # keep the ACT queue clear for the table stage: the gathers
                # (and thus the whole pipeline) wait on its completion
                if last_act[0] is not None:
                    add_dep_helper(
                        inst.ins, last_act[0], sync=False,
                        reason="stats ACT yields to table stage",
                    )
                return inst

            for ti in range(t_tiles):
                sq1 = sm.tile([128, 192], F32, tag="sq1")
                act_after_table(nc.scalar.activation(
                    out=sq1, in_=x_sb[:, ti, 128:320],
                    func=AF.Square, scale=sc1, accum_out=v4[:, ti, 2:3],
                ))
                sq2 = sm.tile([128, 160], F32, tag="sq2")
                act_after_table(nc.scalar.activation(
                    out=sq2, in_=x_sb[:, ti, 320:480],
                    func=AF.Square, scale=sc2, accum_out=v4[:, ti, 3:4],
                ))
            rr = sm.tile([128, t_tiles, 3], F32, tag="rr", bufs=8)
            act_after_table(nc.scalar.activation(
                out=rr, in_=v4[:, :, 1:4], func=AF.Sqrt, bias=eps_sb))
            nc.vector.reciprocal(out=rr, in_=rr)  # rstd0, rnorm1, rnorm2
            state[st] = (x_sb, v4, rr)

        def emit_apply(st):
            x_sb, v4, rr = state.pop(st)
            g = g_tiles.pop(st)
            # gather-dependent per-node multipliers live in the apply phase so
            # a late gather never head-of-line-blocks the next tile's stats
            sp1 = sm.tile([128, t_tiles, 3], F32, tag="sp1", bufs=8)
            nc.vector.tensor_scalar_add(out=sp1, in0=g[:, :, 0:3], scalar1=1.0)
            amul = sm.tile([128, t_tiles, 3], F32, tag="amul", bufs=8)
            nc.vector.tensor_mul(out=amul, in0=rr, in1=sp1)
            bmn = sm.tile([128, t_tiles, 1], F32, tag="bmn", bufs=8)
            nc.vector.tensor_mul(out=bmn, in0=v4[:, :, 0:1], in1=amul[:, :, 0:1])
            # apply IN-PLACE on the x tile: no y buffers, so x slots recycle
            # right after the store and the input DMA streams at wire rate
            if True:
                nc.vector.tensor_tensor(
                    out=x_sb[:, :, 128:320], in0=x_sb[:, :, 128:320],
                    in1=_bcast(amul[:, :, 1:2], 192), op=ALU.mult,
                )
                nc.vector.tensor_tensor(
                    out=x_sb[:, :, 320:480], in0=x_sb[:, :, 320:480],
                    in1=_bcast(amul[:, :, 2:3], 160), op=ALU.mult,
                )
                nc.vector.tensor_tensor(
                    out=x_sb[:, :, 0:128], in0=x_sb[:, :, 0:128],
                    in1=_bcast(amul[:, :, 0:1], 128), op=ALU.mult,
                )
                nc.vector.tensor_tensor(
                    out=x_sb[:, :, 0:128], in0=x_sb[:, :, 0:128],
                    in1=_bcast(bmn[:, :, 0:1], 128), op=ALU.subtract,
                )
            nc.vector.tensor_tensor(
                out=x_sb[:, :, 0:128], in0=x_sb[:, :, 0:128],
                in1=g[:, :, 3:131], op=ALU.add,
            )
            rows = slice(st * stn, (st + 1) * stn)
            nc.sync.dma_start(
                out=out_ext[rows, :].rearrange("(p t) c -> p t c", t=t_tiles),
                in_=x_sb,
            )

        for st in range(n_st + 1):
            if st < n_st:
                emit_stats(st)
            if st >= 1:
                emit_apply(st - 1)

    nc.finalize()  # Bacc.finalize runs compile(): sem legalization, lib loads
    return nc


def _prep_in_maps(node_input, t, batch, w1, b1, w2, b2, wm, bm, n_nodes=PER_CORE,
                  t_tiles=T_TILES):
    stn = t_tiles * 128
    n_st = n_nodes // stn
    wmp = np.zeros((TIME, TBL_W), np.float32)
    wmp[:, 0:3] = wm[:, 0:3]
    wmp[:, 3:131] = wm[:, 224:352]
    bmp = np.zeros((TBL_W,), np.float32)
    bmp[0:3] = bm[0:3]
    bmp[3:131] = bm[224:352]
    shared = {
        "t": np.ascontiguousarray(t, dtype=np.float32),
        "w1": np.ascontiguousarray(w1, dtype=np.float32),
        "b1": np.ascontiguousarray(b1, dtype=np.float32),
        "w2": np.ascontiguousarray(w2, dtype=np.float32),
        "b2": np.ascontiguousarray(b2, dtype=np.float32),
        "wmp": wmp,
        "bmp": bmp,
    }
    n = node_input.shape[0]
    starts = [min(i * n_nodes, n - n_nodes) for i in range(N_CORES)]
    in_maps = []
    for s in starts:
        sl = slice(s, s + n_nodes)
        # node (st, p, j) = base + p*t_tiles + j; dma_gather writes slot
        # i = j*128 + p of super-tile st to [p, st, j, :], and reads slot i's
        # index from idx[(i%16) + 16k, st*stw + i//16].
        ids = batch[sl].astype(np.int16).reshape(n_st, 128, t_tiles)
        perm = ids.transpose(0, 2, 1).reshape(n_st, stn)       # [st, j*128+p]
        # [16, n_st*stw] with element [i%16, st*stw + i//16] = perm[st, i]
        cols = perm.reshape(n_st, stn // 16, 16)               # [st, c, r]
        idx16 = np.concatenate([cols[s2].T for s2 in range(n_st)], axis=1)
        idx = np.ascontiguousarray(np.tile(idx16, (8, 1)))
        in_maps.append(
            {
                **shared,
                "node_input": np.ascontiguousarray(node_input[sl], dtype=np.float32),
                "idx": idx,
            }
        )
    return in_maps, starts


_NC_CACHE: dict = {}


def _get_nc(n_nodes=PER_CORE, t_tiles=T_TILES):
    key = (n_nodes, t_tiles)
    if key not in _NC_CACHE:
        _NC_CACHE[key] = build_nc(n_nodes, t_tiles)
    return _NC_CACHE[key]


def run(node_input, t, batch, w1, b1, w2, b2, wm, bm, trace=False, **trace_kwargs):
    """Run on 8 NeuronCores; returns (full output, BassKernelResults)."""
    node_input = np.asarray(node_input)
    n = node_input.shape[0]
    in_maps, starts = _prep_in_maps(
        node_input, np.asarray(t), np.asarray(batch),
        np.asarray(w1), np.asarray(b1), np.asarray(w2), np.asarray(b2),
        np.asarray(wm), np.asarray(bm),
    )
    nc = _get_nc()
    res = run_bass_kernel_spmd(
        nc, in_maps, core_ids=list(range(N_CORES)), trace=trace, **trace_kwargs
    )
    out = np.empty((n, D_IN), dtype=np.float32)
    for s, core_res in zip(starts, res.results):
        out[s:s + PER_CORE] = core_res["out"]
    return out, res


def kernel(node_input, t, batch, w1, b1, w2, b2, wm, bm):
    out, _ = run(node_input, t, batch, w1, b1, w2, b2, wm, bm, trace=False)
    return out
